# revision 4
# baseline (speedup 1.0000x reference)
"""SupCon cluster-memory loss kernel for 8 TRN2 NeuronCores — raw bass.

Math (per core, N-shard of 1024 bank rows x 4 (anchor, bank) combos):
  sumexp[i] = sum_j exp((x_a . mem_b_j)/T - shift_b)
via fp8 DoubleRow matmuls + ScalarE Exp + VectorE/GpSimd row-sums.
The positives term is host-side index bookkeeping (no device work).

v2 schedule (vs the whole-block baseline):
- All input DMAs are kp-sliced (<=131KB).  Probe-measured HWDGE behavior:
  a <=131KB transfer's completion semaphore lands WITH the data; the
  2.3us final-increment lag only afflicts large back-to-back transfers.
  So the matmul stream starts ~3.5us earlier.
- Three queues: scalar + sync HWDGE rings carry bank-1 slices, x slices
  and shift (consumed first); the gpsimd SWDGE ring carries all bank-0
  slices (needed ~8us later; SWDGE's ~2.6us startup is irrelevant).
- Supergroups de-interleaved (one (b,mt,a) at a time) so sg1 completes
  as early as DMA allows; sg1's exp is split into nt-halves so the
  Scalar engine starts exping at ~10.5us.
- Exp chunks: sg1 halves, sg2..sg7 full, sg8 halves (last half with
  fused row-accum into rs).  Row-reduces are split across Vector and
  GpSimd so no engine backlogs at stream end.
- No trailing drains: the neuronxcc custom-kernel lowering appends its
  own all-engine barrier + drains + a fixed ~6.5us sweep zeroing sems
  3..255 on every execution.
"""

import ml_dtypes
import numpy as np

import concourse.bacc as bacc
import concourse.mybir as mybir
from concourse.bass_utils import run_bass_kernel_spmd

BF16_NP = ml_dtypes.bfloat16

B = 256          # anchor batch per modality
N = 8192         # memory bank rows
D = 768          # feature dim
NCORES = 8
NS = N // NCORES     # 1024 bank rows per core
KT = D // 128        # 6 contraction tiles
MT = B // 128        # 2 anchor partition tiles
SUPCON_T = 0.07

F32 = mybir.dt.float32
FP8 = mybir.dt.float8e4
FP8_NP = ml_dtypes.float8_e4m3
FP8_SCALE = 16.0

NWU = 13         # warmup matmuls ([128, 256] free dim)
NOUT = 10        # rs cols (see CHUNKS)

# Supergroup order (b, mt, a), de-interleaved.
SGS = [(1, 0, 0), (1, 1, 0), (1, 0, 1), (1, 1, 1),
       (0, 0, 0), (0, 1, 0), (0, 0, 1), (0, 1, 1)]
# acc bank per sg (reused after that acc's reduces retire)
SG_ACC = [0, 1, 2, 3, 0, 1, 2, 3]

# Exp chunks: (sg_idx, col_lo, col_hi, rs_col).  s_mm counts chunks in
# this order; reduces follow the same numbering.
CHUNKS = [(0, 0, 512, 0), (0, 512, 1024, 1),
          (1, 0, 1024, 2), (2, 0, 1024, 3), (3, 0, 1024, 4),
          (4, 0, 1024, 5), (5, 0, 1024, 6), (6, 0, 1024, 7),
          (7, 0, 512, 8), (7, 512, 1024, 9)]
# reduce owner per chunk: v=Vector, a=ACT accum (no separate reduce)
RED_OWNER = ['v', 'v', 'v', 'v', 'v', 'v', 'v', 'v', 'a', 'a']

_NC_CACHE = {}


def _build_nc():
    nc = bacc.Bacc("TRN2", target_bir_lowering=False, debug=False,
                   num_devices=NCORES)

    # xT split by anchor half a: [2][128, KT, B] fp8.
    xT = nc.dram_tensor("xT", [2, 128, KT, B], FP8, kind="ExternalInput").ap()
    # memB[b][nt] = [128, KT, 512] fp8 column block of bank b.
    memB = nc.dram_tensor("memB", [2, 2, 128, KT, 512], FP8,
                          kind="ExternalInput").ap()
    nshift_h = nc.dram_tensor("nshift", [128, MT, 2, 2], F32,
                              kind="ExternalInput").ap()
    res = nc.dram_tensor("res", [128, NOUT], F32, kind="ExternalOutput").ap()

    x_sb = [nc.alloc_sbuf_tensor(f"x{a}", [128, KT, B], FP8).ap()
            for a in range(2)]
    blk = {(b, nt): nc.alloc_sbuf_tensor(f"m{b}{nt}", [128, KT, 512], FP8).ap()
           for b in range(2) for nt in range(2)}
    shift_t = nc.alloc_sbuf_tensor("shift", [128, MT, 2, 2], F32).ap()
    rs = nc.alloc_sbuf_tensor("rs", [128, NOUT], F32).ap()
    wu_w = nc.alloc_sbuf_tensor("wu_w", [128, 128], FP8).ap()
    wu_r = nc.alloc_sbuf_tensor("wu_r", [128, 256], FP8).ap()
    acc = [nc.alloc_psum_tensor(f"acc{i}", [128, 1024], F32).ap()
           for i in range(4)]

    def sem(name):
        return nc.alloc_semaphore(name)

    # ---- DMA slice plan.  Each entry: (engine, kind, args) in queue
    # order; kind 'b' = bank slice (b, nt, kp), 'x' = x slice (a, kp),
    # 'xw' = x whole-remainder, 's' = shift.
    q_scalar = [('x', 0, 0), ('b', 1, 0, 1), ('x', 0, 2),
                ('b', 1, 1, 0), ('b', 1, 1, 2)]
    q_sync = [('b', 1, 0, 0), ('s',), ('x', 0, 1), ('b', 1, 0, 2),
              ('b', 1, 1, 1), ('x', 1, 0), ('x', 1, 2), ('x', 1, 1)]
    q_gp = [('b', 0, 0, 0), ('b', 0, 0, 1), ('b', 0, 0, 2),
            ('b', 0, 1, 0), ('b', 0, 1, 1), ('b', 0, 1, 2)]

    bank_sem = {}
    x_sem = {}
    shift_sem = None

    def issue(eng, plan):
        nonlocal shift_sem
        for item in plan:
            if item[0] == 'b':
                _, b, nt, kp = item
                s = sem(f"d_b{b}{nt}{kp}")
                eng.dma_start(
                    out=blk[b, nt][:, 2 * kp:2 * kp + 2],
                    in_=memB[b, nt][:, 2 * kp:2 * kp + 2]).then_inc(s, 16)
                bank_sem[(b, nt, kp)] = s
            elif item[0] == 'x':
                _, a, kp = item
                s = sem(f"d_x{a}{kp}")
                eng.dma_start(
                    out=x_sb[a][:, 2 * kp:2 * kp + 2],
                    in_=xT[a][:, 2 * kp:2 * kp + 2]).then_inc(s, 16)
                x_sem[(a, kp)] = s
            else:
                s = sem("d_shift")
                eng.dma_start(out=shift_t, in_=nshift_h).then_inc(s, 16)
                shift_sem = s

    issue(nc.scalar, q_scalar)
    issue(nc.sync, q_sync)
    issue(nc.gpsimd, q_gp)

    s_mm = sem("s_mm")      # chunk accumulation done (Tensor), CHUNKS order
    s_exp = sem("s_exp")    # chunk exp done (Scalar)
    s_redv = sem("s_redv")  # Vector reduces done
    s_redg = sem("s_redg")  # GpSimd reduces done
    s_acc = sem("s_acc")    # final accum chunk retired (Scalar)
    s_fin = sem("s_fin")    # output DMA

    # Acc WAR gates: sg i (i>=4) reuses acc of sg i-4; it may start once
    # the donor sg's reduces retired.  Donor chunk -> (owner, count):
    # sg0 chunks 0,1 (v#1,2); sg1 chunk 2 (v#3); sg2 chunk 3 (v#4);
    # sg3 chunk 4 (v#5).
    WAR = {4: ('v', 2), 5: ('v', 3), 6: ('v', 4), 7: ('v', 5)}

    # ---- Tensor: warmups then the 48-matmul stream.
    for _ in range(NWU):
        nc.tensor.matmul(acc[0][:, 0:256], wu_w, wu_r, start=True, stop=True)

    hi = {}

    def twait(s, v):
        if hi.get(s.num, 0) < v:
            hi[s.num] = v
            nc.tensor.wait_ge(s, v)

    chunk_i = 0
    for si, (b, mt, a) in enumerate(SGS):
        if si in WAR:
            owner, cnt = WAR[si]
            twait(s_redv if owner == 'v' else s_redg, cnt)
        for nt in range(2):
            for kp in range(KT // 2):
                twait(bank_sem[(b, nt, kp)], 16)
                twait(x_sem[(a, kp)], 16)
                mm = nc.tensor.matmul(
                    acc[SG_ACC[si]][:, nt * 512:(nt + 1) * 512],
                    x_sb[a][:, 2 * kp:2 * kp + 2, mt * 128:(mt + 1) * 128],
                    blk[b, nt][:, 2 * kp:2 * kp + 2],
                    start=(kp == 0), stop=(kp == KT // 2 - 1),
                    perf_mode=mybir.MatmulPerfMode.DoubleRow)
            # chunk boundaries: sg0 halves (after nt0 / nt1), sg1..sg6
            # full (after nt1), sg7 halves.
            if si in (0, 7) or nt == 1:
                chunk_i += 1
                mm.then_inc(s_mm, 1)
    assert chunk_i == len(CHUNKS)

    # ---- Scalar: exps per chunk.
    scale = 1.0 / (SUPCON_T * FP8_SCALE * FP8_SCALE)
    nc.scalar.wait_ge(shift_sem, 16)
    for ci, (si, lo, hi_c, col) in enumerate(CHUNKS):
        b, mt, a = SGS[si]
        bias = shift_t[:, mt, a, b:b + 1]
        nc.scalar.wait_ge(s_mm, ci + 1)
        if RED_OWNER[ci] == 'a':
            nc.scalar.activation(
                out=acc[SG_ACC[si]][:, lo:hi_c],
                in_=acc[SG_ACC[si]][:, lo:hi_c],
                func=mybir.ActivationFunctionType.Exp,
                bias=bias, scale=scale,
                accum_out=rs[:, col:col + 1]).then_inc(s_acc, 1)
        else:
            nc.scalar.activation(
                out=acc[SG_ACC[si]][:, lo:hi_c],
                in_=acc[SG_ACC[si]][:, lo:hi_c],
                func=mybir.ActivationFunctionType.Exp,
                bias=bias, scale=scale).then_inc(s_exp, 1)

    # ---- Vector / GpSimd: row reduces per chunk.
    for ci, (si, lo, hi_c, col) in enumerate(CHUNKS):
        own = RED_OWNER[ci]
        if own == 'a':
            continue
        eng = nc.vector if own == 'v' else nc.gpsimd
        eng.wait_ge(s_exp, ci + 1)
        eng.tensor_reduce(out=rs[:, col:col + 1],
                          in_=acc[SG_ACC[si]][:, lo:hi_c],
                          axis=mybir.AxisListType.X,
                          op=mybir.AluOpType.add).then_inc(
            s_redv if own == 'v' else s_redg, 1)

    # ---- Sync: output DMA once every rs column is written.
    nv = RED_OWNER.count('v')
    ng = RED_OWNER.count('g')
    na = RED_OWNER.count('a')
    nc.sync.wait_ge(s_redv, nv)
    if ng:
        nc.sync.wait_ge(s_redg, ng)
    nc.sync.wait_ge(s_acc, na)
    nc.sync.dma_start(out=res, in_=rs).then_inc(s_fin, 16)

    # No explicit drains/barrier: the neuronxcc custom-kernel lowering
    # appends its own all-engine barrier + drains + sem sweep.

    nc.compile()
    return nc


def get_nc():
    if "nc" not in _NC_CACHE:
        _NC_CACHE["nc"] = _build_nc()
    return _NC_CACHE["nc"]


def _l2norm(x):
    n = np.linalg.norm(x, axis=-1, keepdims=True)
    return x / np.maximum(n, 1e-12)


def _gather_positives(feats_b, lab_a, mlab_b):
    """G[i] = sum of bank rows whose prototype label == lab_a[i]."""
    G = np.zeros((B, D), np.float32)
    if np.unique(mlab_b).size == mlab_b.size:
        inv = np.full(1 << 14, -1, np.int64)
        inv[mlab_b] = np.arange(mlab_b.size)
        idx = inv[np.clip(lab_a, 0, (1 << 14) - 1)]
        valid = idx >= 0
        G[valid] = feats_b[idx[valid]]
    else:
        by_label = np.zeros((1 << 14, D), np.float32)
        np.add.at(by_label, mlab_b, feats_b)
        G[:] = by_label[np.clip(lab_a, 0, (1 << 14) - 1)]
    return G


def make_in_maps(inputs_rgb, inputs_ir, targets_rgb, targets_ir,
                 features_rgb, features_ir,
                 prototype_labels_rgb, prototype_labels_ir):
    x = [_l2norm(np.asarray(inputs_rgb, np.float32)),
         _l2norm(np.asarray(inputs_ir, np.float32))]
    feats = [np.asarray(features_rgb, np.float32),
             np.asarray(features_ir, np.float32)]
    lab = [np.asarray(targets_rgb).astype(np.int64),
           np.asarray(targets_ir).astype(np.int64)]
    mlab = [np.asarray(prototype_labels_rgb).astype(np.int64),
            np.asarray(prototype_labels_ir).astype(np.int64)]

    # xT[a] = [128, KT, B]: x[a].T tiled over kt.
    xT = np.empty([2, 128, KT, B], np.float32)
    for a in range(2):
        xT[a] = (x[a].T.reshape(KT, 128, B) * FP8_SCALE).transpose(1, 0, 2)
    xT = np.ascontiguousarray(xT).astype(FP8_NP)

    bank_max = [float(np.sqrt((feats[b] ** 2).sum(axis=1).max()))
                for b in range(2)]
    shift = np.empty((B, 2, 2), np.float64)                   # [i, a, b]
    if max(bank_max) <= 2.0:
        for b in range(2):
            shift[:, :, b] = bank_max[b] / SUPCON_T
    else:
        for a in range(2):
            for b in range(2):
                shift[:, a, b] = (x[a] @ feats[b].T).max(axis=1) / SUPCON_T
    nshift = np.ascontiguousarray(
        (-shift).reshape(MT, 128, 2, 2).transpose(1, 0, 2, 3)).astype(np.float32)

    # Host-side positives: pos[a][b][i] = x[a][i] . G_ab[i].
    pos = np.empty((2, 2, B), np.float64)
    for a in range(2):
        for b in range(2):
            G = _gather_positives(feats[b], lab[a], mlab[b])
            pos[a, b] = (x[a].astype(np.float64) *
                         G.astype(np.float64)).sum(axis=1)

    in_maps = []
    for c in range(NCORES):
        memB = np.empty([2, 2, 128, KT, 512], FP8_NP)
        for b in range(2):
            for nt in range(2):
                b_rows = feats[b][c * NS + nt * 512:c * NS + (nt + 1) * 512, :]
                memB[b, nt] = (b_rows.T * FP8_SCALE).reshape(
                    KT, 128, 512).transpose(1, 0, 2).astype(FP8_NP)
        in_maps.append({
            "xT": xT,
            "memB": memB,
            "nshift": nshift,
        })
    return in_maps, (shift, pos)


def combine(results, aux, targets_rgb, targets_ir,
            prototype_labels_rgb, prototype_labels_ir):
    shift, pos = aux
    rs = np.stack([np.asarray(r["res"], np.float64) for r in results])
    rs_sum = rs.sum(axis=0)                                    # [128, NOUT]
    sumexp = np.zeros((B, 4), np.float64)
    for ci, (si, lo, hi_c, col) in enumerate(CHUNKS):
        b, mt, a = SGS[si]
        c = a * 2 + b
        sumexp[mt * 128:(mt + 1) * 128, c] += rs_sum[:, col]

    lab = [np.asarray(targets_rgb).astype(np.int64),
           np.asarray(targets_ir).astype(np.int64)]
    mlab = [np.asarray(prototype_labels_rgb).astype(np.int64),
            np.asarray(prototype_labels_ir).astype(np.int64)]

    losses = np.zeros(4, np.float64)
    for a in range(2):
        for b in range(2):
            c = a * 2 + b
            lse = shift[:, a, b] + np.log(sumexp[:, c])
            cnt = np.bincount(mlab[b], minlength=1 << 14)[
                np.clip(lab[a], 0, (1 << 14) - 1)].astype(np.float64)
            mlpp = (pos[a, b] / SUPCON_T - cnt * lse) / np.maximum(cnt, 1.0)
            losses[c] = -mlpp.mean()

    loss_contr = losses[0] + losses[3]        # (rgb,rgb) + (ir,ir)
    loss_cross = losses[1] + losses[2]        # (rgb,ir)  + (ir,rgb)
    return np.asarray([loss_contr, loss_cross], np.float32)


def run_device(in_maps, **kwargs):
    return run_bass_kernel_spmd(get_nc(), in_maps,
                                core_ids=list(range(NCORES)), **kwargs)


def kernel(inputs_rgb, inputs_ir, targets_rgb, targets_ir,
           features_rgb, features_ir,
           prototype_labels_rgb, prototype_labels_ir):
    in_maps, aux = make_in_maps(inputs_rgb, inputs_ir, targets_rgb,
                                targets_ir, features_rgb, features_ir,
                                prototype_labels_rgb, prototype_labels_ir)
    results = run_device(in_maps).results
    return combine(results, aux, targets_rgb, targets_ir,
                   prototype_labels_rgb, prototype_labels_ir)


# revision 6
# speedup vs baseline: 1.0951x; 1.0951x over previous
"""SupCon cluster-memory loss kernel for 8 TRN2 NeuronCores — raw bass.

Math (per core, N-shard of 1024 bank rows x 4 (anchor, bank) combos):
  sumexp[i] = sum_j exp((x_a . mem_b_j)/T - shift_b)
via fp8 DoubleRow matmuls + ScalarE Exp + VectorE/GpSimd row-sums.
The positives term is host-side index bookkeeping (no device work).

v2 schedule (vs the whole-block baseline):
- All input DMAs are kp-sliced (<=131KB).  Probe-measured HWDGE behavior:
  a <=131KB transfer's completion semaphore lands WITH the data; the
  2.3us final-increment lag only afflicts large back-to-back transfers.
  So the matmul stream starts ~3.5us earlier.
- Three queues: scalar + sync HWDGE rings carry bank-1 slices, x slices
  and shift (consumed first); the gpsimd SWDGE ring carries all bank-0
  slices (needed ~8us later; SWDGE's ~2.6us startup is irrelevant).
- Supergroups de-interleaved (one (b,mt,a) at a time) so sg1 completes
  as early as DMA allows; sg1's exp is split into nt-halves so the
  Scalar engine starts exping at ~10.5us.
- Exp chunks: sg1 halves, sg2..sg7 full, sg8 halves (last half with
  fused row-accum into rs).  Row-reduces are split across Vector and
  GpSimd so no engine backlogs at stream end.
- No trailing drains: the neuronxcc custom-kernel lowering appends its
  own all-engine barrier + drains + a fixed ~6.5us sweep zeroing sems
  3..255 on every execution.
"""

import ml_dtypes
import numpy as np

import concourse.bacc as bacc
import concourse.mybir as mybir
from concourse.bass_utils import run_bass_kernel_spmd

BF16_NP = ml_dtypes.bfloat16

B = 256          # anchor batch per modality
N = 8192         # memory bank rows
D = 768          # feature dim
NCORES = 8
NS = N // NCORES     # 1024 bank rows per core
KT = D // 128        # 6 contraction tiles
MT = B // 128        # 2 anchor partition tiles
SUPCON_T = 0.07

F32 = mybir.dt.float32
FP8 = mybir.dt.float8e4
FP8_NP = ml_dtypes.float8_e4m3
FP8_SCALE = 16.0

NWU = 13         # warmup matmuls ([128, 256] free dim)
NOUT = 10        # rs cols (see CHUNKS)

# Supergroup order (b, mt, a), de-interleaved.
SGS = [(1, 0, 0), (1, 1, 0), (1, 0, 1), (1, 1, 1),
       (0, 0, 0), (0, 1, 0), (0, 0, 1), (0, 1, 1)]
# acc bank per sg (reused after that acc's reduces retire)
SG_ACC = [0, 1, 2, 3, 0, 1, 2, 3]

# Exp chunks: (sg_idx, col_lo, col_hi, rs_col).  s_mm counts chunks in
# this order; reduces follow the same numbering.
CHUNKS = [(0, 0, 512, 0), (0, 512, 1024, 1),
          (1, 0, 1024, 2), (2, 0, 1024, 3), (3, 0, 1024, 4),
          (4, 0, 1024, 5), (5, 0, 1024, 6), (6, 0, 1024, 7),
          (7, 0, 512, 8), (7, 512, 1024, 9)]
# reduce owner per chunk: v=Vector, a=ACT accum (no separate reduce)
RED_OWNER = ['v', 'v', 'v', 'v', 'v', 'v', 'v', 'v', 'a', 'a']

_NC_CACHE = {}


def _build_nc():
    nc = bacc.Bacc("TRN2", target_bir_lowering=False, debug=False,
                   num_devices=NCORES)

    # xT split by anchor half a: [2][128, KT, B] fp8.
    xT = nc.dram_tensor("xT", [2, 128, KT, B], FP8, kind="ExternalInput").ap()
    # memB[b][nt] = [128, KT, 512] fp8 column block of bank b.
    memB = nc.dram_tensor("memB", [2, 2, 128, KT, 512], FP8,
                          kind="ExternalInput").ap()
    nshift_h = nc.dram_tensor("nshift", [128, MT, 2, 2], F32,
                              kind="ExternalInput").ap()
    res = nc.dram_tensor("res", [128, NOUT], F32, kind="ExternalOutput").ap()

    x_sb = [nc.alloc_sbuf_tensor(f"x{a}", [128, KT, B], FP8).ap()
            for a in range(2)]
    blk = {(b, nt): nc.alloc_sbuf_tensor(f"m{b}{nt}", [128, KT, 512], FP8).ap()
           for b in range(2) for nt in range(2)}
    shift_t = nc.alloc_sbuf_tensor("shift", [128, MT, 2, 2], F32).ap()
    rs = nc.alloc_sbuf_tensor("rs", [128, NOUT], F32).ap()
    wu_w = nc.alloc_sbuf_tensor("wu_w", [128, 128], FP8).ap()
    wu_r = nc.alloc_sbuf_tensor("wu_r", [128, 256], FP8).ap()
    acc = [nc.alloc_psum_tensor(f"acc{i}", [128, 1024], F32).ap()
           for i in range(4)]

    def sem(name):
        return nc.alloc_semaphore(name)

    # ---- DMA slice plan.  Each entry: (engine, kind, args) in queue
    # order; kind 'b' = bank slice (b, nt, kp), 'x' = x slice (a, kp),
    # 'xw' = x whole-remainder, 's' = shift.
    q_scalar = [('x', 0, 0), ('b', 1, 0, 1), ('x', 0, 2),
                ('b', 1, 1, 0), ('b', 1, 1, 2)]
    q_sync = [('b', 1, 0, 0), ('s',), ('x', 0, 1), ('b', 1, 0, 2),
              ('b', 1, 1, 1), ('x', 1, 0), ('x', 1, 2), ('x', 1, 1),
              ('b', 0, 0, 0), ('b', 0, 1, 0)]
    # gpsimd (SWDGE) is gated on the b1 fill finishing so it doesn't
    # steal HBM bandwidth from the stream-critical front.
    q_gp = [('b', 0, 0, 1), ('b', 0, 1, 1), ('b', 0, 0, 2), ('b', 0, 1, 2)]

    bank_sem = {}
    x_sem = {}
    shift_sem = None

    def issue(eng, plan):
        nonlocal shift_sem
        for item in plan:
            if item[0] == 'b':
                _, b, nt, kp = item
                s = sem(f"d_b{b}{nt}{kp}")
                eng.dma_start(
                    out=blk[b, nt][:, 2 * kp:2 * kp + 2],
                    in_=memB[b, nt][:, 2 * kp:2 * kp + 2]).then_inc(s, 16)
                bank_sem[(b, nt, kp)] = s
            elif item[0] == 'x':
                _, a, kp = item
                s = sem(f"d_x{a}{kp}")
                eng.dma_start(
                    out=x_sb[a][:, 2 * kp:2 * kp + 2],
                    in_=xT[a][:, 2 * kp:2 * kp + 2]).then_inc(s, 16)
                x_sem[(a, kp)] = s
            else:
                s = sem("d_shift")
                eng.dma_start(out=shift_t, in_=nshift_h).then_inc(s, 16)
                shift_sem = s

    issue(nc.scalar, q_scalar)
    issue(nc.sync, q_sync)
    # Gate SWDGE behind the scalar ring's 4th item (b1n1s0): by then the
    # front fill is nearly done and the b0 slices can't starve it.
    nc.gpsimd.wait_ge(bank_sem[(1, 1, 0)], 16)
    issue(nc.gpsimd, q_gp)

    s_mm = sem("s_mm")      # chunk accumulation done (Tensor), CHUNKS order
    s_exp = sem("s_exp")    # chunk exp done (Scalar)
    s_redv = sem("s_redv")  # Vector reduces done
    s_redg = sem("s_redg")  # GpSimd reduces done
    s_acc = sem("s_acc")    # final accum chunk retired (Scalar)
    s_fin = sem("s_fin")    # output DMA

    # Acc WAR gates: sg i (i>=4) reuses acc of sg i-4; it may start once
    # the donor sg's reduces retired.  Donor chunk -> (owner, count):
    # sg0 chunks 0,1 (v#1,2); sg1 chunk 2 (v#3); sg2 chunk 3 (v#4);
    # sg3 chunk 4 (v#5).
    WAR = {4: ('v', 2), 5: ('v', 3), 6: ('v', 4), 7: ('v', 5)}

    # ---- Tensor: warmups then the 48-matmul stream.
    for _ in range(NWU):
        nc.tensor.matmul(acc[0][:, 0:256], wu_w, wu_r, start=True, stop=True)

    hi = {}

    def twait(s, v):
        if hi.get(s.num, 0) < v:
            hi[s.num] = v
            nc.tensor.wait_ge(s, v)

    chunk_i = 0
    for si, (b, mt, a) in enumerate(SGS):
        if si in WAR:
            owner, cnt = WAR[si]
            twait(s_redv if owner == 'v' else s_redg, cnt)
        for nt in range(2):
            for kp in range(KT // 2):
                twait(bank_sem[(b, nt, kp)], 16)
                twait(x_sem[(a, kp)], 16)
                mm = nc.tensor.matmul(
                    acc[SG_ACC[si]][:, nt * 512:(nt + 1) * 512],
                    x_sb[a][:, 2 * kp:2 * kp + 2, mt * 128:(mt + 1) * 128],
                    blk[b, nt][:, 2 * kp:2 * kp + 2],
                    start=(kp == 0), stop=(kp == KT // 2 - 1),
                    perf_mode=mybir.MatmulPerfMode.DoubleRow)
            # chunk boundaries: sg0 halves (after nt0 / nt1), sg1..sg6
            # full (after nt1), sg7 halves.
            if si in (0, 7) or nt == 1:
                chunk_i += 1
                mm.then_inc(s_mm, 1)
    assert chunk_i == len(CHUNKS)

    # ---- Scalar: exps per chunk.
    scale = 1.0 / (SUPCON_T * FP8_SCALE * FP8_SCALE)
    nc.scalar.wait_ge(shift_sem, 16)
    for ci, (si, lo, hi_c, col) in enumerate(CHUNKS):
        b, mt, a = SGS[si]
        bias = shift_t[:, mt, a, b:b + 1]
        nc.scalar.wait_ge(s_mm, ci + 1)
        if RED_OWNER[ci] == 'a':
            nc.scalar.activation(
                out=acc[SG_ACC[si]][:, lo:hi_c],
                in_=acc[SG_ACC[si]][:, lo:hi_c],
                func=mybir.ActivationFunctionType.Exp,
                bias=bias, scale=scale,
                accum_out=rs[:, col:col + 1]).then_inc(s_acc, 1)
        else:
            nc.scalar.activation(
                out=acc[SG_ACC[si]][:, lo:hi_c],
                in_=acc[SG_ACC[si]][:, lo:hi_c],
                func=mybir.ActivationFunctionType.Exp,
                bias=bias, scale=scale).then_inc(s_exp, 1)

    # ---- Vector / GpSimd: row reduces per chunk.
    for ci, (si, lo, hi_c, col) in enumerate(CHUNKS):
        own = RED_OWNER[ci]
        if own == 'a':
            continue
        eng = nc.vector if own == 'v' else nc.gpsimd
        eng.wait_ge(s_exp, ci + 1)
        eng.tensor_reduce(out=rs[:, col:col + 1],
                          in_=acc[SG_ACC[si]][:, lo:hi_c],
                          axis=mybir.AxisListType.X,
                          op=mybir.AluOpType.add).then_inc(
            s_redv if own == 'v' else s_redg, 1)

    # ---- Sync: output DMA once every rs column is written.
    nv = RED_OWNER.count('v')
    ng = RED_OWNER.count('g')
    na = RED_OWNER.count('a')
    nc.sync.wait_ge(s_redv, nv)
    if ng:
        nc.sync.wait_ge(s_redg, ng)
    nc.sync.wait_ge(s_acc, na)
    nc.sync.dma_start(out=res, in_=rs).then_inc(s_fin, 16)

    # No explicit drains/barrier: the neuronxcc custom-kernel lowering
    # appends its own all-engine barrier + drains + sem sweep.

    nc.compile()
    return nc


def get_nc():
    if "nc" not in _NC_CACHE:
        _NC_CACHE["nc"] = _build_nc()
    return _NC_CACHE["nc"]


def _l2norm(x):
    n = np.linalg.norm(x, axis=-1, keepdims=True)
    return x / np.maximum(n, 1e-12)


def _gather_positives(feats_b, lab_a, mlab_b):
    """G[i] = sum of bank rows whose prototype label == lab_a[i]."""
    G = np.zeros((B, D), np.float32)
    if np.unique(mlab_b).size == mlab_b.size:
        inv = np.full(1 << 14, -1, np.int64)
        inv[mlab_b] = np.arange(mlab_b.size)
        idx = inv[np.clip(lab_a, 0, (1 << 14) - 1)]
        valid = idx >= 0
        G[valid] = feats_b[idx[valid]]
    else:
        by_label = np.zeros((1 << 14, D), np.float32)
        np.add.at(by_label, mlab_b, feats_b)
        G[:] = by_label[np.clip(lab_a, 0, (1 << 14) - 1)]
    return G


def make_in_maps(inputs_rgb, inputs_ir, targets_rgb, targets_ir,
                 features_rgb, features_ir,
                 prototype_labels_rgb, prototype_labels_ir):
    x = [_l2norm(np.asarray(inputs_rgb, np.float32)),
         _l2norm(np.asarray(inputs_ir, np.float32))]
    feats = [np.asarray(features_rgb, np.float32),
             np.asarray(features_ir, np.float32)]
    lab = [np.asarray(targets_rgb).astype(np.int64),
           np.asarray(targets_ir).astype(np.int64)]
    mlab = [np.asarray(prototype_labels_rgb).astype(np.int64),
            np.asarray(prototype_labels_ir).astype(np.int64)]

    # xT[a] = [128, KT, B]: x[a].T tiled over kt.
    xT = np.empty([2, 128, KT, B], np.float32)
    for a in range(2):
        xT[a] = (x[a].T.reshape(KT, 128, B) * FP8_SCALE).transpose(1, 0, 2)
    xT = np.ascontiguousarray(xT).astype(FP8_NP)

    bank_max = [float(np.sqrt((feats[b] ** 2).sum(axis=1).max()))
                for b in range(2)]
    shift = np.empty((B, 2, 2), np.float64)                   # [i, a, b]
    if max(bank_max) <= 2.0:
        for b in range(2):
            shift[:, :, b] = bank_max[b] / SUPCON_T
    else:
        for a in range(2):
            for b in range(2):
                shift[:, a, b] = (x[a] @ feats[b].T).max(axis=1) / SUPCON_T
    nshift = np.ascontiguousarray(
        (-shift).reshape(MT, 128, 2, 2).transpose(1, 0, 2, 3)).astype(np.float32)

    # Host-side positives: pos[a][b][i] = x[a][i] . G_ab[i].
    pos = np.empty((2, 2, B), np.float64)
    for a in range(2):
        for b in range(2):
            G = _gather_positives(feats[b], lab[a], mlab[b])
            pos[a, b] = (x[a].astype(np.float64) *
                         G.astype(np.float64)).sum(axis=1)

    in_maps = []
    for c in range(NCORES):
        memB = np.empty([2, 2, 128, KT, 512], FP8_NP)
        for b in range(2):
            for nt in range(2):
                b_rows = feats[b][c * NS + nt * 512:c * NS + (nt + 1) * 512, :]
                memB[b, nt] = (b_rows.T * FP8_SCALE).reshape(
                    KT, 128, 512).transpose(1, 0, 2).astype(FP8_NP)
        in_maps.append({
            "xT": xT,
            "memB": memB,
            "nshift": nshift,
        })
    return in_maps, (shift, pos)


def combine(results, aux, targets_rgb, targets_ir,
            prototype_labels_rgb, prototype_labels_ir):
    shift, pos = aux
    rs = np.stack([np.asarray(r["res"], np.float64) for r in results])
    rs_sum = rs.sum(axis=0)                                    # [128, NOUT]
    sumexp = np.zeros((B, 4), np.float64)
    for ci, (si, lo, hi_c, col) in enumerate(CHUNKS):
        b, mt, a = SGS[si]
        c = a * 2 + b
        sumexp[mt * 128:(mt + 1) * 128, c] += rs_sum[:, col]

    lab = [np.asarray(targets_rgb).astype(np.int64),
           np.asarray(targets_ir).astype(np.int64)]
    mlab = [np.asarray(prototype_labels_rgb).astype(np.int64),
            np.asarray(prototype_labels_ir).astype(np.int64)]

    losses = np.zeros(4, np.float64)
    for a in range(2):
        for b in range(2):
            c = a * 2 + b
            lse = shift[:, a, b] + np.log(sumexp[:, c])
            cnt = np.bincount(mlab[b], minlength=1 << 14)[
                np.clip(lab[a], 0, (1 << 14) - 1)].astype(np.float64)
            mlpp = (pos[a, b] / SUPCON_T - cnt * lse) / np.maximum(cnt, 1.0)
            losses[c] = -mlpp.mean()

    loss_contr = losses[0] + losses[3]        # (rgb,rgb) + (ir,ir)
    loss_cross = losses[1] + losses[2]        # (rgb,ir)  + (ir,rgb)
    return np.asarray([loss_contr, loss_cross], np.float32)


def run_device(in_maps, **kwargs):
    return run_bass_kernel_spmd(get_nc(), in_maps,
                                core_ids=list(range(NCORES)), **kwargs)


def kernel(inputs_rgb, inputs_ir, targets_rgb, targets_ir,
           features_rgb, features_ir,
           prototype_labels_rgb, prototype_labels_ir):
    in_maps, aux = make_in_maps(inputs_rgb, inputs_ir, targets_rgb,
                                targets_ir, features_rgb, features_ir,
                                prototype_labels_rgb, prototype_labels_ir)
    results = run_device(in_maps).results
    return combine(results, aux, targets_rgb, targets_ir,
                   prototype_labels_rgb, prototype_labels_ir)
